# revision 8
# baseline (speedup 1.0000x reference)
"""Trainium2 Bass kernel for nn_ConduitHydrology (MFD flow accumulation).

The reference graph is the raster 4-neighbor grid on a 1024x1024 raster, so
all segment_sums are 5-point stencil operations. Strategy:
  - Row-partition the grid across 8 cores: core k owns global rows
    [128k, 128k+128). Each core computes on a 192-row slab (32-row halo on
    each side); 32 Jacobi iterations x 1-hop stencil means the halo fully
    absorbs cross-partition influence -> zero inter-core communication.
  - On-chip layout: columns -> partitions (col = c*128 + p for chunk c in
    [0,8)), rows packed in the free dim with stride 256 per chunk
    (f = c*256 + r). Row shifts = free-dim AP offsets; column shifts = PE
    shift-matmuls accumulating in PSUM; chunk seams = extra matmuls with
    single-entry matrices.
  - Per iteration: 4 elementwise weight*discharge products (DVE/GpSimd),
    7 matmul groups on PE into PSUM (runoff + 4 shifted inflows + seams),
    and an ACT copy PSUM->SBUF for the new discharge.
The host only pads/slices/relayouts numpy arrays (no arithmetic on host).
"""

import numpy as np

import concourse.bass as bass
import concourse.mybir as mybir
from concourse.bacc import Bacc
from concourse.tile import TileContext
from concourse.bass_utils import run_bass_kernel_spmd

F32 = mybir.dt.float32
I32 = mybir.dt.int32
ALU = mybir.AluOpType
ACTF = mybir.ActivationFunctionType

ROWS = COLS = 1024
N_CORES = 8
N_ITERS = 32
P = 128          # partitions
NCH = 8          # column chunks
CST = 256        # chunk stride in free dim
FD = NCH * CST   # 2048
RQ = 192         # q-domain rows per slab (128 owned + 2*32 halo)
RS = 194         # phi-domain rows (q-domain + 1 extra row each side)
OWN = 128        # owned rows per core
OWN0 = 32        # q-domain row index of first owned row

RHO_W, GRAV, SEC_PER_A = 1000.0, 9.81, 31556926.0
FLOW_COEFF = 0.0405

PAD_BED = 1.0e30  # pad phi ~ 1e34: kills fake drops into the pad rows


def _v(t, base, cnt, c0=0, ncs=NCH):
    """[p, chunks c0:c0+ncs, rows base:base+cnt] view of a [128, 2048] tile."""
    return t.rearrange("p (c r) -> p c r", c=NCH)[:, c0 : c0 + ncs, base : base + cnt]


def build(n_iters=N_ITERS):
    nc = Bacc(None)

    bed_d = nc.declare_dram_parameter("bed", [P, FD], F32, isOutput=False)
    press_d = nc.declare_dram_parameter("press", [P, FD], F32, isOutput=False)
    status_d = nc.declare_dram_parameter("status", [P, FD], I32, isOutput=False)
    melt_d = nc.declare_dram_parameter("melt", [P, FD], F32, isOutput=False)
    area_d = nc.declare_dram_parameter("area", [P, FD], F32, isOutput=False)
    cond_d = nc.declare_dram_parameter("conduit", [P, FD], F32, isOutput=False)
    mats_d = nc.declare_dram_parameter("mats", [P, 896], F32, isOutput=False)
    grad_d = nc.declare_dram_parameter("grad", [P, 1024], F32, isOutput=True)

    with TileContext(nc) as tc:
        with (
            tc.tile_pool(name="main", bufs=1) as pool,
            tc.tile_pool(name="ps", bufs=2, space="PSUM") as pspool,
        ):
            def tmp(tag):
                return pool.tile([P, FD], F32, tag=tag, name=tag)

            def psum():
                return pspool.tile([P, FD], F32, tag="ps", name="ps")

            def emit_group(ops):
                """ops: list of (chunk, out_ap, lhsT, rhs_ap). Emits per-chunk
                matmuls; start=True on the first matmul touching each PSUM
                bank (clears the whole bank's has_written bits), stop=True on
                the last."""
                last = {}
                for i, (c, *_r) in enumerate(ops):
                    last[c // 2] = i
                seen = set()
                for i, (c, o, w, rh) in enumerate(ops):
                    b = c // 2
                    st = b not in seen
                    seen.add(b)
                    nc.tensor.matmul(o, w, rh, start=st, stop=(last[b] == i))

            # ---- load constants & inputs
            mats = pool.tile([P, 896], F32)
            nc.sync.dma_start(out=mats[:], in_=mats_d[:])
            ID = mats[:, 0:128]
            SHD = mats[:, 128:256]   # out[m] = rhs[m-1]
            SHU = mats[:, 256:384]   # out[m] = rhs[m+1]
            EDN = mats[:, 384:512]   # out[0] = rhs[127]
            EUP = mats[:, 512:640]   # out[127] = rhs[0]
            FIXC = mats[:, 640:896]  # row 0 = 1e33 (fake E neighbor of col 1023)

            bed = tmp("t0")
            press = tmp("t1")
            status = pool.tile([P, FD], I32, tag="t2", name="t2")
            melt = tmp("t3")
            area = tmp("t4")
            cond = pool.tile([P, FD], F32)
            for t, d in ((bed, bed_d), (press, press_d), (status, status_d),
                         (melt, melt_d), (area, area_d), (cond, cond_d)):
                nc.sync.dma_start(out=t[:], in_=d[:])

            # ---- runoff (q-domain rows 0..191); melt/area die here
            r = pool.tile([P, FD], F32)
            nc.vector.scalar_tensor_tensor(
                out=_v(r, 0, RQ), in0=_v(melt, 0, RQ), scalar=1.0 / SEC_PER_A,
                in1=_v(area, 0, RQ), op0=ALU.mult, op1=ALU.mult)

            # ---- hydraulic potential and core mask (phi-domain rows 0..193)
            phi = tmp("t5")
            nc.vector.scalar_tensor_tensor(
                out=_v(phi, 0, RS), in0=_v(bed, 0, RS), scalar=RHO_W * GRAV,
                in1=_v(press, 0, RS), op0=ALU.mult, op1=ALU.add)
            m = pool.tile([P, FD], F32)
            nc.vector.tensor_scalar(
                out=_v(m, 0, RS), in0=_v(status, 0, RS), scalar1=0,
                scalar2=None, op0=ALU.is_equal)

            # ---- E-neighbor phi and mask (value at col+1):
            #      within-chunk partition shift (SHU) + seam
            #      (p127 of chunk c <- p0 of chunk c+1).
            def shift_from_east(dst, src, fix=None):
                ps = psum()
                ops = [(c, _v(ps, 0, RS, c, 1), SHU, _v(src, 0, RS, c, 1))
                       for c in range(NCH)]
                ops += [(c, _v(ps, 0, RS, c, 1), EUP, _v(src, 0, RS, c + 1, 1))
                        for c in range(NCH - 1)]
                if fix is not None:
                    # p127 of the last chunk: fake E neighbor from a constant
                    ops.append((NCH - 1, _v(ps, 0, RS, NCH - 1, 1), EUP,
                                fix[:, 0:RS]))
                emit_group(ops)
                nc.scalar.copy(_v(dst, 0, RS), _v(ps, 0, RS))

            # global col 1023 has no E neighbor: give it a fake one at
            # +1e33 so relu(phi - 1e33) == 0; its mask shifts in as 0
            # naturally (zero column in SHU).
            phiE = tmp("t3")      # reuses melt slot
            shift_from_east(phiE, phi, fix=FIXC)
            mE = tmp("t4")        # reuses area slot
            shift_from_east(mE, m)

            # ---- directional drops (link grids, phi-domain)
            dphiE = tmp("t0")     # reuses bed slot
            nc.vector.tensor_sub(_v(dphiE, 0, RS), _v(phi, 0, RS), _v(phiE, 0, RS))
            dropE = tmp("t1")     # flow col -> col+1, stored at col
            nc.vector.scalar_tensor_tensor(
                out=_v(dropE, 0, RS), in0=_v(dphiE, 0, RS), scalar=0.0,
                in1=_v(m, 0, RS), op0=ALU.max, op1=ALU.mult)
            tw = tmp("t3")
            nc.vector.tensor_scalar(
                out=_v(tw, 0, RS), in0=_v(dphiE, 0, RS), scalar1=-1.0,
                scalar2=0.0, op0=ALU.mult, op1=ALU.max)
            dropW = pool.tile([P, FD], F32, tag="t2", name="t2f")
            nc.vector.tensor_mul(_v(dropW, 0, RS), _v(tw, 0, RS), _v(mE, 0, RS))

            dphiS = tmp("t4")     # phi[r] - phi[r+1], link at r
            nc.vector.tensor_sub(_v(dphiS, 0, RS - 1), _v(phi, 0, RS - 1),
                                 _v(phi, 1, RS - 1))
            dropS = tmp("t6")     # flow r -> r+1, stored at r
            nc.vector.scalar_tensor_tensor(
                out=_v(dropS, 0, RS - 1), in0=_v(dphiS, 0, RS - 1), scalar=0.0,
                in1=_v(m, 0, RS - 1), op0=ALU.max, op1=ALU.mult)
            tn = tmp("t3")
            nc.vector.tensor_scalar(
                out=_v(tn, 0, RS - 1), in0=_v(dphiS, 0, RS - 1), scalar1=-1.0,
                scalar2=0.0, op0=ALU.mult, op1=ALU.max)
            dropN = tmp("t7")     # flow r+1 -> r, stored at r
            nc.vector.tensor_mul(_v(dropN, 0, RS - 1), _v(tn, 0, RS - 1),
                                 _v(m, 1, RS - 1))

            # ---- outgoing-W drop gathered at its source node (q-domain):
            #      dW[p] = dropW[p-1] (+ seam p0 of chunk c <- p127 of c-1).
            psW = psum()
            ops = [(c, _v(psW, 0, RQ, c, 1), SHD, _v(dropW, 1, RQ, c, 1))
                   for c in range(NCH)]
            ops += [(c, _v(psW, 0, RQ, c, 1), EDN, _v(dropW, 1, RQ, c - 1, 1))
                    for c in range(1, NCH)]
            emit_group(ops)
            dW = tmp("t3")
            nc.scalar.copy(_v(dW, 0, RQ), _v(psW, 0, RQ))

            # ---- total outgoing drop per node (q-domain)
            psT = psum()
            ops = []
            for c in range(NCH):
                o = _v(psT, 0, RQ, c, 1)
                ops += [(c, o, ID, _v(dropE, 1, RQ, c, 1)),
                        (c, o, ID, _v(dropS, 1, RQ, c, 1)),
                        (c, o, ID, _v(dropN, 0, RQ, c, 1)),
                        (c, o, ID, _v(dW, 0, RQ, c, 1))]
            emit_group(ops)
            tds = tmp("t0")
            nc.vector.tensor_scalar(
                out=_v(tds, 0, RQ), in0=_v(psT, 0, RQ), scalar1=1.0e-30,
                scalar2=None, op0=ALU.max)
            recip = tmp("t4")
            nc.vector.reciprocal(_v(recip, 0, RQ), _v(tds, 0, RQ))

            # ---- outflow fractions (q-domain, at the source node)
            fE = pool.tile([P, FD], F32)
            fW = pool.tile([P, FD], F32)
            fS = pool.tile([P, FD], F32)
            fN = pool.tile([P, FD], F32)
            nc.vector.tensor_mul(_v(fE, 0, RQ), _v(dropE, 1, RQ), _v(recip, 0, RQ))
            nc.vector.tensor_mul(_v(fW, 0, RQ), _v(dW, 0, RQ), _v(recip, 0, RQ))
            nc.vector.tensor_mul(_v(fS, 0, RQ), _v(dropS, 1, RQ), _v(recip, 0, RQ))
            nc.vector.tensor_mul(_v(fN, 0, RQ), _v(dropN, 0, RQ), _v(recip, 0, RQ))

            # ---- initial discharge
            q = pool.tile([P, FD], F32)
            nc.scalar.copy(_v(q, 0, RQ), _v(r, 0, RQ))

            oE = pool.tile([P, FD], F32)
            oW = pool.tile([P, FD], F32)
            oS = pool.tile([P, FD], F32)
            oN = pool.tile([P, FD], F32)

            # ---- fixed-point iterations
            for _ in range(n_iters):
                # 16 bank-local products; 11 on DVE, 5 on GpSimd (~2x slower)
                for b in range(4):
                    eng = nc.vector if b < 4 else nc.gpsimd
                    eng.tensor_mul(_v(oE, 0, RQ, 2 * b, 2),
                                   _v(fE, 0, RQ, 2 * b, 2), _v(q, 0, RQ, 2 * b, 2))
                for b in range(4):
                    eng = nc.gpsimd if b >= 2 else nc.vector
                    eng.tensor_mul(_v(oW, 0, RQ, 2 * b, 2),
                                   _v(fW, 0, RQ, 2 * b, 2), _v(q, 0, RQ, 2 * b, 2))
                for b in range(4):
                    eng = nc.gpsimd if b == 0 else nc.vector
                    eng.tensor_mul(_v(oS, 0, RQ, 2 * b, 2),
                                   _v(fS, 0, RQ, 2 * b, 2), _v(q, 0, RQ, 2 * b, 2))
                for b in range(4):
                    eng = nc.gpsimd if b >= 2 else nc.vector
                    eng.tensor_mul(_v(oN, 0, RQ, 2 * b, 2),
                                   _v(fN, 0, RQ, 2 * b, 2), _v(q, 0, RQ, 2 * b, 2))

                ps = psum()
                ops = []
                # G0: runoff
                ops += [(c, _v(ps, 0, RQ, c, 1), ID, _v(r, 0, RQ, c, 1))
                        for c in range(NCH)]
                # G1: inflow from W neighbor = oE shifted one col east
                ops += [(c, _v(ps, 0, RQ, c, 1), SHD, _v(oE, 0, RQ, c, 1))
                        for c in range(NCH)]
                # G2: seam (chunk c p0 <- chunk c-1 p127)
                ops += [(c, _v(ps, 0, RQ, c, 1), EDN, _v(oE, 0, RQ, c - 1, 1))
                        for c in range(1, NCH)]
                # G3: inflow from E neighbor = oW shifted one col west
                ops += [(c, _v(ps, 0, RQ, c, 1), SHU, _v(oW, 0, RQ, c, 1))
                        for c in range(NCH)]
                # G4: seam (chunk c p127 <- chunk c+1 p0)
                ops += [(c, _v(ps, 0, RQ, c, 1), EUP, _v(oW, 0, RQ, c + 1, 1))
                        for c in range(NCH - 1)]
                # G5: inflow from N neighbor = oS shifted one row down
                ops += [(c, _v(ps, 1, RQ - 1, c, 1), ID, _v(oS, 0, RQ - 1, c, 1))
                        for c in range(NCH)]
                # G6: inflow from S neighbor = oN shifted one row up
                ops += [(c, _v(ps, 0, RQ - 1, c, 1), ID, _v(oN, 1, RQ - 1, c, 1))
                        for c in range(NCH)]
                emit_group(ops)
                # new discharge
                for b in range(4):
                    nc.scalar.copy(_v(q, 0, RQ, 2 * b, 2), _v(ps, 0, RQ, 2 * b, 2))

            # ---- hydraulic gradient on owned rows:
            #      g = (q * FLOW_COEFF * conduit^1.25)^2 * core_mask
            s1 = tmp("t0")
            nc.scalar.sqrt(_v(s1, 0, OWN), _v(cond, 0, OWN))
            s2 = tmp("t1")
            nc.scalar.sqrt(_v(s2, 0, OWN), _v(s1, 0, OWN))
            c125 = tmp("t3")
            nc.vector.tensor_mul(_v(c125, 0, OWN), _v(cond, 0, OWN), _v(s2, 0, OWN))
            k0 = tmp("t4")
            nc.scalar.activation(_v(k0, 0, OWN), _v(c125, 0, OWN), ACTF.Square,
                                 scale=float(FLOW_COEFF))
            km = tmp("t5")
            nc.vector.tensor_mul(_v(km, 0, OWN), _v(k0, 0, OWN),
                                 _v(m, OWN0 + 1, OWN))
            q2 = tmp("t6")
            nc.scalar.activation(_v(q2, 0, OWN), _v(q, OWN0, OWN), ACTF.Square)
            g = tmp("t7")
            nc.vector.tensor_mul(_v(g, 0, OWN), _v(q2, 0, OWN), _v(km, 0, OWN))

            nc.sync.dma_start(
                out=grad_d[:].rearrange("p (c j) -> p c j", c=NCH),
                in_=_v(g, 0, OWN))

    nc.finalize()
    return nc


# ------------------------------------------------------------------ host side

def _mats():
    ident = np.eye(P, dtype=np.float32)
    shd = np.zeros((P, P), np.float32)
    shd[np.arange(P - 1), np.arange(1, P)] = 1.0      # out[m] = rhs[m-1]
    shu = np.zeros((P, P), np.float32)
    shu[np.arange(1, P), np.arange(P - 1)] = 1.0      # out[m] = rhs[m+1]
    edn = np.zeros((P, P), np.float32)
    edn[P - 1, 0] = 1.0                               # out[0] = rhs[127]
    eup = np.zeros((P, P), np.float32)
    eup[0, P - 1] = 1.0                               # out[127] = rhs[0]
    fixc = np.zeros((P, 2 * P), np.float32)
    fixc[0, :] = 1.0e33
    return np.concatenate([ident, shd, shu, edn, eup, fixc], axis=1)


def _to_dev(slab):
    """[rows<=256, 1024] row-major slab -> [128, 2048] chunked layout."""
    rows = slab.shape[0]
    out = np.zeros((P, NCH, CST), dtype=slab.dtype)
    out[:, :, :rows] = slab.reshape(rows, NCH, P).transpose(2, 1, 0)
    return out.reshape(P, FD)


_BUILT = None


def _get_built():
    global _BUILT
    if _BUILT is None:
        _BUILT = build()
    return _BUILT


def _make_in_maps(melt_rate, bedrock_elevation, water_pressure, cell_area,
                  conduit_size, status_at_node):
    grid = lambda a: np.asarray(a).reshape(ROWS, COLS)
    bed = grid(bedrock_elevation).astype(np.float32)
    press = grid(water_pressure).astype(np.float32)
    status = grid(status_at_node).astype(np.int32)
    melt = grid(melt_rate).astype(np.float32)
    area = grid(cell_area).astype(np.float32)
    cond = grid(conduit_size).astype(np.float32)

    gp = 33
    bedp = np.full((ROWS + 2 * gp, COLS), PAD_BED, np.float32)
    bedp[gp:gp + ROWS] = bed
    pressp = np.zeros((ROWS + 2 * gp, COLS), np.float32)
    pressp[gp:gp + ROWS] = press
    statusp = np.ones((ROWS + 2 * gp, COLS), np.int32)
    statusp[gp:gp + ROWS] = status
    gq = 32
    meltp = np.zeros((ROWS + 2 * gq, COLS), np.float32)
    meltp[gq:gq + ROWS] = melt
    areap = np.zeros((ROWS + 2 * gq, COLS), np.float32)
    areap[gq:gq + ROWS] = area

    mats = _mats()
    in_maps = []
    for k in range(N_CORES):
        r0 = k * OWN
        in_maps.append({
            "bed": _to_dev(bedp[r0 : r0 + RS]),
            "press": _to_dev(pressp[r0 : r0 + RS]),
            "status": _to_dev(statusp[r0 : r0 + RS]),
            "melt": _to_dev(meltp[r0 : r0 + RQ]),
            "area": _to_dev(areap[r0 : r0 + RQ]),
            "conduit": _to_dev(cond[r0 : r0 + OWN]),
            "mats": mats,
        })
    return in_maps


def _from_dev(res_maps):
    out = np.empty((ROWS, COLS), np.float32)
    for k in range(N_CORES):
        g = res_maps[k]["grad"].reshape(P, NCH, P)      # [p, c, j]
        out[k * OWN : (k + 1) * OWN] = g.transpose(2, 1, 0).reshape(OWN, COLS)
    return out.ravel()


def run(inputs, trace=False, **kwargs):
    nc = _get_built()
    in_maps = _make_in_maps(
        inputs["melt_rate"], inputs["bedrock_elevation"],
        inputs["water_pressure"], inputs["cell_area"],
        inputs["conduit_size"], inputs["status_at_node"])
    res = run_bass_kernel_spmd(nc, in_maps, list(range(N_CORES)),
                               trace=trace, **kwargs)
    return _from_dev(res.results), res


def kernel(**inputs):
    out, _ = run(inputs)
    return out


# revision 10
# speedup vs baseline: 1.3857x; 1.3857x over previous
"""Trainium2 Bass kernel for nn_ConduitHydrology (MFD flow accumulation).

The reference graph is the raster 4-neighbor grid on a 1024x1024 raster, so
all segment_sums are 5-point stencil operations. Strategy:
  - Row-partition across 8 cores: core k owns global rows [128k, 128k+128),
    computing on a 192-row slab (32-row halo each side). 32 Jacobi
    iterations x 1-hop stencil => the halo fully absorbs cross-partition
    influence: zero inter-core communication.
  - On-chip layout (interleaved): column = p*8 + c for partition p, chunk
    c in [0,8); rows packed contiguously per chunk (f = c*192 + r for the
    q-domain, c*194 + r for the phi-domain). Row shifts and 7/8 of column
    shifts are free-dim offsets; only the chunk seam (c=7 <-> c=0 of the
    next partition) needs a partition-shift matmul.
  - Per iteration: 8 half-width fp16 products (DVE+GpSimd), 26 fp16
    matmuls on PE accumulating all shifted inflows into fp32 PSUM
    (24 of them with the identity as stationary), and 4 DVE adds
    (fp32 PSUM + fp32 runoff -> fp16 q). The last iteration assembles
    fp32 q for the output math.
The host only pads/slices/relayouts numpy arrays (no arithmetic on host).
"""

import numpy as np

import concourse.bass as bass
import concourse.mybir as mybir
from concourse.bacc import Bacc
from concourse.tile import TileContext
from concourse.bass_utils import run_bass_kernel_spmd

F32 = mybir.dt.float32
F16 = mybir.dt.float16
I32 = mybir.dt.int32
ALU = mybir.AluOpType
ACTF = mybir.ActivationFunctionType

ROWS = COLS = 1024
N_CORES = 8
N_ITERS = 32
P = 128
NCH = 8
RQ = 192          # q-domain rows per slab
RS = 194          # phi-domain rows per slab
FQ = NCH * RQ     # 1536
FS = NCH * RS     # 1552
OWN = 128
OWN0 = 32

RHO_W, GRAV, SEC_PER_A = 1000.0, 9.81, 31556926.0
FLOW_COEFF = 0.0405
PAD_BED = 1.0e30


def build(n_iters=N_ITERS):
    nc = Bacc(None)

    bed_d = nc.declare_dram_parameter("bed", [P, FS], F32, isOutput=False)
    press_d = nc.declare_dram_parameter("press", [P, FS], F32, isOutput=False)
    status_d = nc.declare_dram_parameter("status", [P, FS], I32, isOutput=False)
    melt_d = nc.declare_dram_parameter("melt", [P, FQ], F32, isOutput=False)
    area_d = nc.declare_dram_parameter("area", [P, FQ], F32, isOutput=False)
    cond_d = nc.declare_dram_parameter("conduit", [P, 1024], F32, isOutput=False)
    mats_d = nc.declare_dram_parameter("mats", [P, 896], F32, isOutput=False)
    grad_d = nc.declare_dram_parameter("grad", [P, 1024], F32, isOutput=True)

    # phi-domain / q-domain chunk slices (1D)
    sch = lambda t, c, b, n: t[:, c * RS + b : c * RS + b + n]
    qch = lambda t, c, b, n: t[:, c * RQ + b : c * RQ + b + n]
    # 2D chunked views
    vs = lambda t, b, n: t.rearrange("p (c r) -> p c r", c=NCH)[:, :, b : b + n]
    vq = vs

    # iteration PSUM layout: chunk c at f = 512*(c//2) + 192*(c%2)
    pcf = lambda c: 512 * (c // 2) + 192 * (c % 2)
    # setup PSUM layout: chunk c at f = 256*c
    scf = lambda c: 256 * c

    with TileContext(nc) as tc:
        with (
            tc.tile_pool(name="main", bufs=1) as pool,
            tc.tile_pool(name="ps", bufs=2, space="PSUM") as pspool,
        ):
            def tmp(tag):
                return pool.tile([P, FS], F32, tag=tag, name=tag)

            def psum():
                return pspool.tile([P, 2048], F32, tag="ps", name="ps")

            def emit_group(ops):
                """ops: (out_ap, lhsT, rhs_ap, bank). start=True on the first
                matmul touching each PSUM bank (must cover the bank's used
                region), stop on the last."""
                last = {}
                for i, (o, w, rh, bank) in enumerate(ops):
                    last[bank] = i
                seen = set()
                for i, (o, w, rh, bank) in enumerate(ops):
                    st = bank not in seen
                    seen.add(bank)
                    nc.tensor.matmul(o, w, rh, start=st, stop=(last[bank] == i))

            # ---- constants
            mats = pool.tile([P, 896], F32)
            nc.sync.dma_start(out=mats[:], in_=mats_d[:])
            ID = mats[:, 0:128]
            SHD = mats[:, 128:256]   # out[m] = rhs[m-1]
            SHU = mats[:, 256:384]   # out[m] = rhs[m+1]
            EUP = mats[:, 512:640]   # out[127] = rhs[0]
            FIXC = mats[:, 640:896]  # row 0 = 1e33
            mats16 = pool.tile([P, 384], F16)
            nc.vector.tensor_copy(out=mats16[:], in_=mats[:, 0:384])
            ID16 = mats16[:, 0:128]
            SHD16 = mats16[:, 128:256]
            SHU16 = mats16[:, 256:384]

            # ---- inputs
            bed = tmp("t0")
            press = tmp("t1")
            status = pool.tile([P, FS], I32, tag="t2", name="t2")
            melt = tmp("t3")
            area = tmp("t4")
            cond = pool.tile([P, 1024], F32)
            for t, d, n in ((bed, bed_d, FS), (press, press_d, FS),
                            (status, status_d, FS), (melt, melt_d, FQ),
                            (area, area_d, FQ), (cond, cond_d, 1024)):
                nc.sync.dma_start(out=t[:, 0:n], in_=d[:])

            # ---- runoff (q-domain, fp32)
            r = pool.tile([P, FQ], F32)
            nc.vector.scalar_tensor_tensor(
                out=r[:], in0=melt[:, 0:FQ], scalar=1.0 / SEC_PER_A,
                in1=area[:, 0:FQ], op0=ALU.mult, op1=ALU.mult)

            # ---- potential and core mask (phi-domain)
            phi = tmp("t5")
            nc.vector.scalar_tensor_tensor(
                out=phi[:], in0=bed[:], scalar=RHO_W * GRAV,
                in1=press[:], op0=ALU.mult, op1=ALU.add)
            m = pool.tile([P, FS], F32)
            nc.vector.tensor_scalar(
                out=m[:], in0=status[:], scalar1=0, scalar2=None,
                op0=ALU.is_equal)

            # ---- E-neighbor phi / mask. E neighbor of (p,c): (p,c+1) for
            #      c<7, (p+1, chunk 0) for c=7 (seam); none at (p127,c7).
            def shift_from_east(dst, src, fix=None):
                ps = psum()
                ops = [(ps[:, scf(c) : scf(c) + RS], ID, sch(src, c + 1, 0, RS),
                        c // 2) for c in range(NCH - 1)]
                ops.append((ps[:, scf(7) : scf(7) + RS], SHU, sch(src, 0, 0, RS), 3))
                if fix is not None:
                    ops.append((ps[:, scf(7) : scf(7) + RS], EUP, fix[:, 0:RS], 3))
                emit_group(ops)
                nc.scalar.copy(vs(dst, 0, RS),
                               ps.rearrange("p (c r) -> p c r", c=8)[:, :, 0:RS])

            phiE = tmp("t3")
            shift_from_east(phiE, phi, fix=FIXC)
            mE = tmp("t4")
            shift_from_east(mE, m)

            # ---- directional drops (phi-domain link grids)
            dphiE = tmp("t0")
            nc.vector.tensor_sub(dphiE[:], phi[:], phiE[:])
            dropE = tmp("t1")    # flow col -> col+1, stored at col
            nc.vector.scalar_tensor_tensor(
                out=dropE[:], in0=dphiE[:], scalar=0.0, in1=m[:],
                op0=ALU.max, op1=ALU.mult)
            tw = tmp("t3")
            nc.vector.tensor_scalar(
                out=tw[:], in0=dphiE[:], scalar1=-1.0, scalar2=0.0,
                op0=ALU.mult, op1=ALU.max)
            dropW = pool.tile([P, FS], F32, tag="t2", name="t2f")
            nc.vector.tensor_mul(dropW[:], tw[:], mE[:])

            dphiS = tmp("t4")    # phi[r] - phi[r+1], link at r (per chunk)
            nc.vector.tensor_sub(vs(dphiS, 0, RS - 1), vs(phi, 0, RS - 1),
                                 vs(phi, 1, RS - 1))
            dropS = tmp("t6")    # flow r -> r+1, stored at r
            nc.vector.scalar_tensor_tensor(
                out=vs(dropS, 0, RS - 1), in0=vs(dphiS, 0, RS - 1), scalar=0.0,
                in1=vs(m, 0, RS - 1), op0=ALU.max, op1=ALU.mult)
            tn = tmp("t3")
            nc.vector.tensor_scalar(
                out=vs(tn, 0, RS - 1), in0=vs(dphiS, 0, RS - 1), scalar1=-1.0,
                scalar2=0.0, op0=ALU.mult, op1=ALU.max)
            dropN = tmp("t7")    # flow r+1 -> r, stored at r
            nc.vector.tensor_mul(vs(dropN, 0, RS - 1), vs(tn, 0, RS - 1),
                                 vs(m, 1, RS - 1))

            # ---- outgoing-W drop at its source (q-domain):
            #      dW[p,c] = dropW[(p,c-1)] | dropW[(p-1, c7)]
            psW = psum()
            ops = [(psW[:, scf(c) : scf(c) + RQ], ID, sch(dropW, c - 1, 1, RQ),
                    c // 2) for c in range(1, NCH)]
            ops.append((psW[:, scf(0) : scf(0) + RQ], SHD, sch(dropW, 7, 1, RQ), 0))
            emit_group(ops)
            dW = pool.tile([P, FQ], F32, tag="t3", name="t3w")
            nc.scalar.copy(vq(dW, 0, RQ),
                           psW.rearrange("p (c r) -> p c r", c=8)[:, :, 0:RQ])

            # ---- total outgoing drop (q-domain)
            psT = psum()
            ops = []
            for c in range(NCH):
                o = psT[:, scf(c) : scf(c) + RQ]
                ops += [(o, ID, sch(dropE, c, 1, RQ), c // 2),
                        (o, ID, sch(dropS, c, 1, RQ), c // 2),
                        (o, ID, sch(dropN, c, 0, RQ), c // 2),
                        (o, ID, qch(dW, c, 0, RQ), c // 2)]
            emit_group(ops)
            tds = pool.tile([P, FQ], F32, tag="t0", name="t0t")
            nc.vector.tensor_scalar(
                out=vq(tds, 0, RQ),
                in0=psT.rearrange("p (c r) -> p c r", c=8)[:, :, 0:RQ],
                scalar1=1.0e-30, scalar2=None, op0=ALU.max)
            recip = pool.tile([P, FQ], F32, tag="t4", name="t4r")
            nc.vector.reciprocal(recip[:], tds[:])

            # ---- outflow fractions, cast to fp16 (q-domain, source node)
            fE = pool.tile([P, FQ], F16)
            fW = pool.tile([P, FQ], F16)
            fS = pool.tile([P, FQ], F16)
            fN = pool.tile([P, FQ], F16)
            nc.vector.tensor_mul(vq(fE, 0, RQ), vs(dropE, 1, RQ), vq(recip, 0, RQ))
            nc.vector.tensor_mul(fW[:], dW[:], recip[:])
            nc.vector.tensor_mul(vq(fS, 0, RQ), vs(dropS, 1, RQ), vq(recip, 0, RQ))
            nc.vector.tensor_mul(vq(fN, 0, RQ), vs(dropN, 0, RQ), vq(recip, 0, RQ))

            # ---- discharge iteration state
            q16 = pool.tile([P, FQ], F16)
            nc.scalar.copy(q16[:], r[:])
            q32 = pool.tile([P, FQ], F32)
            oE = pool.tile([P, FQ], F16)
            oW = pool.tile([P, FQ], F16)
            oS = pool.tile([P, FQ], F16)
            oN = pool.tile([P, FQ], F16)

            H = FQ // 2
            for it in range(n_iters):
                lastit = it == n_iters - 1
                # products: DVE takes oW/oE halves, GpSimd takes oS/oN
                for h in (0, 1):
                    sl = slice(h * H, (h + 1) * H)
                    nc.vector.tensor_mul(oW[:, sl], fW[:, sl], q16[:, sl])
                for h in (0, 1):
                    sl = slice(h * H, (h + 1) * H)
                    nc.vector.tensor_mul(oE[:, sl], fE[:, sl], q16[:, sl])
                for h in (0, 1):
                    sl = slice(h * H, (h + 1) * H)
                    nc.gpsimd.tensor_mul(oS[:, sl], fS[:, sl], q16[:, sl])
                for h in (0, 1):
                    sl = slice(h * H, (h + 1) * H)
                    nc.gpsimd.tensor_mul(oN[:, sl], fN[:, sl], q16[:, sl])

                ps = psum()
                ops = []
                # Bank starters first: each must cover its bank's full used
                # region (the PSUM has_written clear makes the first write
                # an overwrite; later sub-range writes accumulate).
                # G3: inflow from E neighbor (oW at (p,c+1); seam (p+1,c0)->c7)
                ops += [(ps[:, 0:384], ID16, oW[:, 192:576], 0),
                        (ps[:, 512:896], ID16, oW[:, 576:960], 1),
                        (ps[:, 1024:1408], ID16, oW[:, 960:1344], 2)]
                # G1 b3 piece covers [1536:1920) -> bank-3 starter
                ops += [(ps[:, 1536:1920], ID16, oE[:, 960:1344], 3),
                        (ps[:, 1536:1728], ID16, oW[:, 1344:1536], 3)]
                # G1: inflow from W neighbor (oE at (p,c-1); seam (p-1,c7)->c0)
                ops += [(ps[:, 192:384], ID16, oE[:, 0:192], 0),
                        (ps[:, 512:896], ID16, oE[:, 192:576], 1),
                        (ps[:, 1024:1408], ID16, oE[:, 576:960], 2)]
                # G5: inflow from N neighbor (row r-1)
                ops += [(ps[:, pcf(c) + 1 : pcf(c) + RQ], ID16,
                         qch(oS, c, 0, RQ - 1), c // 2) for c in range(NCH)]
                # G6: inflow from S neighbor (row r+1)
                ops += [(ps[:, pcf(c) : pcf(c) + RQ - 1], ID16,
                         qch(oN, c, 1, RQ - 1), c // 2) for c in range(NCH)]
                # seams
                ops.append((ps[:, 0:192], SHD16, oE[:, 1344:1536], 0))
                ops.append((ps[:, 1728:1920], SHU16, oW[:, 0:192], 3))
                emit_group(ops)

                # q = PSUM + runoff (fp32 exact); fp16 except final iter
                qdst = q32 if lastit else q16
                for b in range(4):
                    nc.vector.tensor_add(
                        out=qdst[:, 384 * b : 384 * b + 384],
                        in0=ps[:, 512 * b : 512 * b + 384],
                        in1=r[:, 384 * b : 384 * b + 384])

            # ---- gradient on owned rows (compact [p, c*128+j] layout)
            s1 = pool.tile([P, 1024], F32, tag="f0", name="f0")
            nc.scalar.sqrt(s1[:], cond[:])
            s2 = pool.tile([P, 1024], F32, tag="f1", name="f1")
            nc.scalar.sqrt(s2[:], s1[:])
            c125 = pool.tile([P, 1024], F32, tag="f0", name="f0b")
            nc.vector.tensor_mul(c125[:], cond[:], s2[:])
            k0 = pool.tile([P, 1024], F32, tag="f1", name="f1b")
            nc.scalar.activation(k0[:], c125[:], ACTF.Square,
                                 scale=float(FLOW_COEFF))
            vo = lambda t: t.rearrange("p (c j) -> p c j", c=NCH)
            km = pool.tile([P, 1024], F32, tag="f0", name="f0c")
            nc.vector.tensor_mul(vo(km), vo(k0), vs(m, OWN0 + 1, OWN))
            q2 = pool.tile([P, 1024], F32, tag="f1", name="f1c")
            nc.scalar.activation(vo(q2), vq(q32, OWN0, OWN), ACTF.Square)
            g = pool.tile([P, 1024], F32, tag="f2", name="f2")
            nc.vector.tensor_mul(g[:], q2[:], km[:])

            nc.sync.dma_start(out=grad_d[:], in_=g[:])

    nc.finalize()
    return nc


# ------------------------------------------------------------------ host side

def _mats():
    ident = np.eye(P, dtype=np.float32)
    shd = np.zeros((P, P), np.float32)
    shd[np.arange(P - 1), np.arange(1, P)] = 1.0      # out[m] = rhs[m-1]
    shu = np.zeros((P, P), np.float32)
    shu[np.arange(1, P), np.arange(P - 1)] = 1.0      # out[m] = rhs[m+1]
    edn = np.zeros((P, P), np.float32)
    edn[P - 1, 0] = 1.0
    eup = np.zeros((P, P), np.float32)
    eup[0, P - 1] = 1.0
    fixc = np.zeros((P, 2 * P), np.float32)
    fixc[0, :] = 1.0e33
    return np.concatenate([ident, shd, shu, edn, eup, fixc], axis=1)


def _to_dev(slab):
    """[rows, 1024] row-major slab -> [128, 8*rows], col = p*8 + c."""
    rows = slab.shape[0]
    return np.ascontiguousarray(
        slab.reshape(rows, P, NCH).transpose(1, 2, 0)).reshape(P, NCH * rows)


_BUILT = None


def _get_built():
    global _BUILT
    if _BUILT is None:
        _BUILT = build()
    return _BUILT


def _make_in_maps(melt_rate, bedrock_elevation, water_pressure, cell_area,
                  conduit_size, status_at_node):
    grid = lambda a: np.asarray(a).reshape(ROWS, COLS)
    bed = grid(bedrock_elevation).astype(np.float32)
    press = grid(water_pressure).astype(np.float32)
    status = grid(status_at_node).astype(np.int32)
    melt = grid(melt_rate).astype(np.float32)
    area = grid(cell_area).astype(np.float32)
    cond = grid(conduit_size).astype(np.float32)

    gp = 33
    bedp = np.full((ROWS + 2 * gp, COLS), PAD_BED, np.float32)
    bedp[gp:gp + ROWS] = bed
    pressp = np.zeros((ROWS + 2 * gp, COLS), np.float32)
    pressp[gp:gp + ROWS] = press
    statusp = np.ones((ROWS + 2 * gp, COLS), np.int32)
    statusp[gp:gp + ROWS] = status
    gq = 32
    meltp = np.zeros((ROWS + 2 * gq, COLS), np.float32)
    meltp[gq:gq + ROWS] = melt
    areap = np.zeros((ROWS + 2 * gq, COLS), np.float32)
    areap[gq:gq + ROWS] = area

    mats = _mats()
    in_maps = []
    for k in range(N_CORES):
        r0 = k * OWN
        in_maps.append({
            "bed": _to_dev(bedp[r0 : r0 + RS]),
            "press": _to_dev(pressp[r0 : r0 + RS]),
            "status": _to_dev(statusp[r0 : r0 + RS]),
            "melt": _to_dev(meltp[r0 : r0 + RQ]),
            "area": _to_dev(areap[r0 : r0 + RQ]),
            "conduit": _to_dev(cond[r0 : r0 + OWN]),
            "mats": mats,
        })
    return in_maps


def _from_dev(res_maps):
    out = np.empty((ROWS, COLS), np.float32)
    for k in range(N_CORES):
        g = res_maps[k]["grad"].reshape(P, NCH, OWN)    # [p, c, j]
        out[k * OWN : (k + 1) * OWN] = g.transpose(2, 0, 1).reshape(OWN, COLS)
    return out.ravel()


def run(inputs, trace=False, **kwargs):
    nc = _get_built()
    in_maps = _make_in_maps(
        inputs["melt_rate"], inputs["bedrock_elevation"],
        inputs["water_pressure"], inputs["cell_area"],
        inputs["conduit_size"], inputs["status_at_node"])
    res = run_bass_kernel_spmd(nc, in_maps, list(range(N_CORES)),
                               trace=trace, **kwargs)
    return _from_dev(res.results), res


def kernel(**inputs):
    out, _ = run(inputs)
    return out


# revision 12
# speedup vs baseline: 1.6298x; 1.1762x over previous
"""Trainium2 Bass kernel for nn_ConduitHydrology (MFD flow accumulation).

The reference graph is the raster 4-neighbor grid on a 1024x1024 raster, so
all segment_sums are 5-point stencil operations. Strategy:
  - Row-partition across 8 cores: core k owns global rows [128k, 128k+128),
    computing on a 192-row slab (32-row halo each side). 32 Jacobi
    iterations x 1-hop stencil => the halo fully absorbs cross-partition
    influence: zero inter-core communication.
  - On-chip layout (interleaved): column = p*8 + c for partition p, chunk
    c in [0,8); rows packed contiguously per chunk (f = c*192 + r for the
    q-domain, c*194 + r for the phi-domain). Row shifts and 7/8 of column
    shifts are free-dim offsets; only the chunk seam (c=7 <-> c=0 of the
    next partition) needs a partition-shift matmul.
  - Per iteration: 8 half-width fp16 products (DVE+GpSimd), 26 fp16
    matmuls on PE accumulating all shifted inflows into fp32 PSUM
    (24 of them with the identity as stationary), and 4 DVE adds
    (fp32 PSUM + fp32 runoff -> fp16 q). The last iteration assembles
    fp32 q for the output math.
The host only pads/slices/relayouts numpy arrays (no arithmetic on host).
"""

import numpy as np

import concourse.bass as bass
import concourse.mybir as mybir
from concourse.bacc import Bacc
from concourse.tile import TileContext
from concourse.bass_utils import run_bass_kernel_spmd

F32 = mybir.dt.float32
F16 = mybir.dt.bfloat16
I32 = mybir.dt.int32
ALU = mybir.AluOpType
ACTF = mybir.ActivationFunctionType

ROWS = COLS = 1024
N_CORES = 8
N_ITERS = 32
P = 128
NCH = 8
RQ = 192          # q-domain rows per slab
RS = 194          # phi-domain rows per slab
FQ = NCH * RQ     # 1536
FS = NCH * RS     # 1552
OWN = 128
OWN0 = 32

RHO_W, GRAV, SEC_PER_A = 1000.0, 9.81, 31556926.0
FLOW_COEFF = 0.0405
PAD_BED = 1.0e30


def build(n_iters=N_ITERS):
    nc = Bacc(None)

    bed_d = nc.declare_dram_parameter("bed", [P, FS], F32, isOutput=False)
    press_d = nc.declare_dram_parameter("press", [P, FS], F32, isOutput=False)
    status_d = nc.declare_dram_parameter("status", [P, FS], I32, isOutput=False)
    melt_d = nc.declare_dram_parameter("melt", [P, FQ], F32, isOutput=False)
    area_d = nc.declare_dram_parameter("area", [P, FQ], F32, isOutput=False)
    cond_d = nc.declare_dram_parameter("conduit", [P, 1024], F32, isOutput=False)
    mats_d = nc.declare_dram_parameter("mats", [P, 896], F32, isOutput=False)
    grad_d = nc.declare_dram_parameter("grad", [P, 1024], F32, isOutput=True)

    # phi-domain / q-domain chunk slices (1D)
    sch = lambda t, c, b, n: t[:, c * RS + b : c * RS + b + n]
    qch = lambda t, c, b, n: t[:, c * RQ + b : c * RQ + b + n]
    # 2D chunked views
    vs = lambda t, b, n: t.rearrange("p (c r) -> p c r", c=NCH)[:, :, b : b + n]
    vq = vs

    # iteration PSUM layout: chunk c at f = 512*(c//2) + 192*(c%2)
    pcf = lambda c: 512 * (c // 2) + 192 * (c % 2)
    # setup PSUM layout: chunk c at f = 256*c
    scf = lambda c: 256 * c

    with TileContext(nc) as tc:
        with (
            tc.tile_pool(name="main", bufs=1) as pool,
            tc.tile_pool(name="ps", bufs=2, space="PSUM") as pspool,
        ):
            def tmp(tag):
                return pool.tile([P, FS], F32, tag=tag, name=tag)

            def psum():
                return pspool.tile([P, 2048], F32, tag="ps", name="ps")

            def emit_group(ops):
                """ops: (out_ap, lhsT, rhs_ap, bank). start=True on the first
                matmul touching each PSUM bank (must cover the bank's used
                region), stop on the last."""
                last = {}
                for i, (o, w, rh, bank) in enumerate(ops):
                    last[bank] = i
                seen = set()
                for i, (o, w, rh, bank) in enumerate(ops):
                    st = bank not in seen
                    seen.add(bank)
                    nc.tensor.matmul(o, w, rh, start=st, stop=(last[bank] == i))

            # ---- constants
            mats = pool.tile([P, 896], F32)
            nc.sync.dma_start(out=mats[:], in_=mats_d[:])
            ID = mats[:, 0:128]
            SHD = mats[:, 128:256]   # out[m] = rhs[m-1]
            SHU = mats[:, 256:384]   # out[m] = rhs[m+1]
            EUP = mats[:, 512:640]   # out[127] = rhs[0]
            FIXC = mats[:, 640:896]  # row 0 = 1e33
            mats16 = pool.tile([P, 384], F16)
            nc.vector.tensor_copy(out=mats16[:], in_=mats[:, 0:384])
            ID16 = mats16[:, 0:128]
            SHD16 = mats16[:, 128:256]
            SHU16 = mats16[:, 256:384]

            # ---- inputs
            bed = tmp("t0")
            press = tmp("t1")
            status = pool.tile([P, FS], I32, tag="t2", name="t2")
            melt = tmp("t3")
            area = tmp("t4")
            cond = pool.tile([P, 1024], F32)
            for t, d, n in ((bed, bed_d, FS), (press, press_d, FS),
                            (status, status_d, FS), (melt, melt_d, FQ),
                            (area, area_d, FQ), (cond, cond_d, 1024)):
                nc.sync.dma_start(out=t[:, 0:n], in_=d[:])

            # ---- runoff (q-domain, fp32)
            r = pool.tile([P, FQ], F32)
            nc.vector.scalar_tensor_tensor(
                out=r[:], in0=melt[:, 0:FQ], scalar=1.0 / SEC_PER_A,
                in1=area[:, 0:FQ], op0=ALU.mult, op1=ALU.mult)

            # ---- potential and core mask (phi-domain)
            phi = tmp("t5")
            nc.vector.scalar_tensor_tensor(
                out=phi[:], in0=bed[:], scalar=RHO_W * GRAV,
                in1=press[:], op0=ALU.mult, op1=ALU.add)
            m = pool.tile([P, FS], F32)
            nc.vector.tensor_scalar(
                out=m[:], in0=status[:], scalar1=0, scalar2=None,
                op0=ALU.is_equal)

            # ---- E-neighbor phi / mask. E neighbor of (p,c): (p,c+1) for
            #      c<7, (p+1, chunk 0) for c=7 (seam); none at (p127,c7).
            def shift_from_east(dst, src, fix=None):
                ps = psum()
                ops = [(ps[:, scf(c) : scf(c) + RS], ID, sch(src, c + 1, 0, RS),
                        c // 2) for c in range(NCH - 1)]
                ops.append((ps[:, scf(7) : scf(7) + RS], SHU, sch(src, 0, 0, RS), 3))
                if fix is not None:
                    ops.append((ps[:, scf(7) : scf(7) + RS], EUP, fix[:, 0:RS], 3))
                emit_group(ops)
                nc.scalar.copy(vs(dst, 0, RS),
                               ps.rearrange("p (c r) -> p c r", c=8)[:, :, 0:RS])

            phiE = tmp("t3")
            shift_from_east(phiE, phi, fix=FIXC)
            mE = tmp("t4")
            shift_from_east(mE, m)

            # ---- directional drops (phi-domain link grids)
            dphiE = tmp("t0")
            nc.vector.tensor_sub(dphiE[:], phi[:], phiE[:])
            dropE = tmp("t1")    # flow col -> col+1, stored at col
            nc.vector.scalar_tensor_tensor(
                out=dropE[:], in0=dphiE[:], scalar=0.0, in1=m[:],
                op0=ALU.max, op1=ALU.mult)
            tw = tmp("t3")
            nc.vector.tensor_scalar(
                out=tw[:], in0=dphiE[:], scalar1=-1.0, scalar2=0.0,
                op0=ALU.mult, op1=ALU.max)
            dropW = pool.tile([P, FS], F32, tag="t2", name="t2f")
            nc.vector.tensor_mul(dropW[:], tw[:], mE[:])

            dphiS = tmp("t4")    # phi[r] - phi[r+1], link at r (per chunk)
            nc.vector.tensor_sub(vs(dphiS, 0, RS - 1), vs(phi, 0, RS - 1),
                                 vs(phi, 1, RS - 1))
            dropS = tmp("t6")    # flow r -> r+1, stored at r
            nc.vector.scalar_tensor_tensor(
                out=vs(dropS, 0, RS - 1), in0=vs(dphiS, 0, RS - 1), scalar=0.0,
                in1=vs(m, 0, RS - 1), op0=ALU.max, op1=ALU.mult)
            tn = tmp("t3")
            nc.vector.tensor_scalar(
                out=vs(tn, 0, RS - 1), in0=vs(dphiS, 0, RS - 1), scalar1=-1.0,
                scalar2=0.0, op0=ALU.mult, op1=ALU.max)
            dropN = tmp("t7")    # flow r+1 -> r, stored at r
            nc.vector.tensor_mul(vs(dropN, 0, RS - 1), vs(tn, 0, RS - 1),
                                 vs(m, 1, RS - 1))

            # ---- outgoing-W drop at its source (q-domain):
            #      dW[p,c] = dropW[(p,c-1)] | dropW[(p-1, c7)]
            psW = psum()
            ops = [(psW[:, scf(c) : scf(c) + RQ], ID, sch(dropW, c - 1, 1, RQ),
                    c // 2) for c in range(1, NCH)]
            ops.append((psW[:, scf(0) : scf(0) + RQ], SHD, sch(dropW, 7, 1, RQ), 0))
            emit_group(ops)
            dW = pool.tile([P, FQ], F32, tag="t3", name="t3w")
            nc.scalar.copy(vq(dW, 0, RQ),
                           psW.rearrange("p (c r) -> p c r", c=8)[:, :, 0:RQ])

            # ---- total outgoing drop (q-domain)
            psT = psum()
            ops = []
            for c in range(NCH):
                o = psT[:, scf(c) : scf(c) + RQ]
                ops += [(o, ID, sch(dropE, c, 1, RQ), c // 2),
                        (o, ID, sch(dropS, c, 1, RQ), c // 2),
                        (o, ID, sch(dropN, c, 0, RQ), c // 2),
                        (o, ID, qch(dW, c, 0, RQ), c // 2)]
            emit_group(ops)
            tds = pool.tile([P, FQ], F32, tag="t0", name="t0t")
            nc.vector.tensor_scalar(
                out=vq(tds, 0, RQ),
                in0=psT.rearrange("p (c r) -> p c r", c=8)[:, :, 0:RQ],
                scalar1=1.0e-30, scalar2=None, op0=ALU.max)
            recip = pool.tile([P, FQ], F32, tag="t4", name="t4r")
            nc.vector.reciprocal(recip[:], tds[:])

            # ---- outflow fractions, cast to fp16 (q-domain, source node)
            fE = pool.tile([P, FQ], F16)
            fW = pool.tile([P, FQ], F16)
            fS = pool.tile([P, FQ], F16)
            fN = pool.tile([P, FQ], F16)
            nc.vector.tensor_mul(vq(fE, 0, RQ), vs(dropE, 1, RQ), vq(recip, 0, RQ))
            nc.vector.tensor_mul(fW[:], dW[:], recip[:])
            nc.vector.tensor_mul(vq(fS, 0, RQ), vs(dropS, 1, RQ), vq(recip, 0, RQ))
            nc.vector.tensor_mul(vq(fN, 0, RQ), vs(dropN, 0, RQ), vq(recip, 0, RQ))

            # slab-edge outflow rows leave the slab; zero them so the
            # pair-merged row-shift matmuls bleed exact zeros across the
            # chunk boundary inside each PSUM bank.
            nc.vector.memset(vq(fS, RQ - 1, 1), 0.0)
            nc.vector.memset(vq(fN, 0, 1), 0.0)

            # ---- discharge iteration state
            q16 = pool.tile([P, FQ], F16)
            nc.scalar.copy(q16[:], r[:])
            q32 = pool.tile([P, FQ], F32)
            oE = pool.tile([P, FQ], F16)
            oW = pool.tile([P, FQ], F16)
            oS = pool.tile([P, FQ], F16)
            oN = pool.tile([P, FQ], F16)

            H = FQ // 2
            for it in range(n_iters):
                lastit = it == n_iters - 1
                # products: DVE takes oW/oE halves, GpSimd takes oS/oN.
                # Order feeds the PE bank sequence: bank 0 needs oW h0,
                # oE h0+h1 (seam), oS/oN h0.
                for h in (0, 1):
                    sl = slice(h * H, (h + 1) * H)
                    nc.vector.tensor_mul(oW[:, sl], fW[:, sl], q16[:, sl])
                nc.vector.tensor_mul(oE[:, H:FQ], fE[:, H:FQ], q16[:, H:FQ])
                nc.vector.tensor_mul(oE[:, 0:H], fE[:, 0:H], q16[:, 0:H])
                for h in (0, 1):
                    sl = slice(h * H, (h + 1) * H)
                    nc.gpsimd.tensor_mul(oS[:, sl], fS[:, sl], q16[:, sl])
                    nc.gpsimd.tensor_mul(oN[:, sl], fN[:, sl], q16[:, sl])

                ps = psum()
                qdst = q32 if lastit else q16
                # Per-bank, in order: starter (covers the bank's whole used
                # region), accumulators, then the q assembly for that bank
                # so DVE drains banks while PE works on later ones.
                bank_ops = [
                    [   # bank 0: chunks 0,1
                        (ps[:, 0:384], ID16, oW[:, 192:576], 0),
                        (ps[:, 192:384], ID16, oE[:, 0:192], 0),
                        (ps[:, 0:192], SHD16, oE[:, 1344:1536], 0),
                        (ps[:, 1:384], ID16, oS[:, 0:383], 0),
                        (ps[:, 0:383], ID16, oN[:, 1:384], 0),
                    ],
                    [   # bank 1: chunks 2,3
                        (ps[:, 512:896], ID16, oW[:, 576:960], 1),
                        (ps[:, 512:896], ID16, oE[:, 192:576], 1),
                        (ps[:, 513:896], ID16, oS[:, 384:767], 1),
                        (ps[:, 512:895], ID16, oN[:, 385:768], 1),
                    ],
                    [   # bank 2: chunks 4,5
                        (ps[:, 1024:1408], ID16, oW[:, 960:1344], 2),
                        (ps[:, 1024:1408], ID16, oE[:, 576:960], 2),
                        (ps[:, 1025:1408], ID16, oS[:, 768:1151], 2),
                        (ps[:, 1024:1407], ID16, oN[:, 769:1152], 2),
                    ],
                    [   # bank 3: chunks 6,7
                        (ps[:, 1536:1920], ID16, oE[:, 960:1344], 3),
                        (ps[:, 1536:1728], ID16, oW[:, 1344:1536], 3),
                        (ps[:, 1728:1920], SHU16, oW[:, 0:192], 3),
                        (ps[:, 1537:1920], ID16, oS[:, 1152:1535], 3),
                        (ps[:, 1536:1919], ID16, oN[:, 1153:1536], 3),
                    ],
                ]
                for b in range(4):
                    for i, (o, w, rh, _bk) in enumerate(bank_ops[b]):
                        nc.tensor.matmul(o, w, rh, start=(i == 0),
                                         stop=(i == len(bank_ops[b]) - 1))
                    nc.vector.tensor_add(
                        out=qdst[:, 384 * b : 384 * b + 384],
                        in0=ps[:, 512 * b : 512 * b + 384],
                        in1=r[:, 384 * b : 384 * b + 384])

            # ---- gradient on owned rows (compact [p, c*128+j] layout)
            s1 = pool.tile([P, 1024], F32, tag="f0", name="f0")
            nc.scalar.sqrt(s1[:], cond[:])
            s2 = pool.tile([P, 1024], F32, tag="f1", name="f1")
            nc.scalar.sqrt(s2[:], s1[:])
            c125 = pool.tile([P, 1024], F32, tag="f0", name="f0b")
            nc.vector.tensor_mul(c125[:], cond[:], s2[:])
            k0 = pool.tile([P, 1024], F32, tag="f1", name="f1b")
            nc.scalar.activation(k0[:], c125[:], ACTF.Square,
                                 scale=float(FLOW_COEFF))
            vo = lambda t: t.rearrange("p (c j) -> p c j", c=NCH)
            km = pool.tile([P, 1024], F32, tag="f0", name="f0c")
            nc.vector.tensor_mul(vo(km), vo(k0), vs(m, OWN0 + 1, OWN))
            q2 = pool.tile([P, 1024], F32, tag="f1", name="f1c")
            nc.scalar.activation(vo(q2), vq(q32, OWN0, OWN), ACTF.Square)
            g = pool.tile([P, 1024], F32, tag="f2", name="f2")
            nc.vector.tensor_mul(g[:], q2[:], km[:])

            nc.sync.dma_start(out=grad_d[:], in_=g[:])

    nc.finalize()
    return nc


# ------------------------------------------------------------------ host side

def _mats():
    ident = np.eye(P, dtype=np.float32)
    shd = np.zeros((P, P), np.float32)
    shd[np.arange(P - 1), np.arange(1, P)] = 1.0      # out[m] = rhs[m-1]
    shu = np.zeros((P, P), np.float32)
    shu[np.arange(1, P), np.arange(P - 1)] = 1.0      # out[m] = rhs[m+1]
    edn = np.zeros((P, P), np.float32)
    edn[P - 1, 0] = 1.0
    eup = np.zeros((P, P), np.float32)
    eup[0, P - 1] = 1.0
    fixc = np.zeros((P, 2 * P), np.float32)
    fixc[0, :] = 1.0e33
    return np.concatenate([ident, shd, shu, edn, eup, fixc], axis=1)


def _to_dev(slab):
    """[rows, 1024] row-major slab -> [128, 8*rows], col = p*8 + c."""
    rows = slab.shape[0]
    return np.ascontiguousarray(
        slab.reshape(rows, P, NCH).transpose(1, 2, 0)).reshape(P, NCH * rows)


_BUILT = None


def _get_built():
    global _BUILT
    if _BUILT is None:
        _BUILT = build()
    return _BUILT


def _make_in_maps(melt_rate, bedrock_elevation, water_pressure, cell_area,
                  conduit_size, status_at_node):
    grid = lambda a: np.asarray(a).reshape(ROWS, COLS)
    bed = grid(bedrock_elevation).astype(np.float32)
    press = grid(water_pressure).astype(np.float32)
    status = grid(status_at_node).astype(np.int32)
    melt = grid(melt_rate).astype(np.float32)
    area = grid(cell_area).astype(np.float32)
    cond = grid(conduit_size).astype(np.float32)

    gp = 33
    bedp = np.full((ROWS + 2 * gp, COLS), PAD_BED, np.float32)
    bedp[gp:gp + ROWS] = bed
    pressp = np.zeros((ROWS + 2 * gp, COLS), np.float32)
    pressp[gp:gp + ROWS] = press
    statusp = np.ones((ROWS + 2 * gp, COLS), np.int32)
    statusp[gp:gp + ROWS] = status
    gq = 32
    meltp = np.zeros((ROWS + 2 * gq, COLS), np.float32)
    meltp[gq:gq + ROWS] = melt
    areap = np.zeros((ROWS + 2 * gq, COLS), np.float32)
    areap[gq:gq + ROWS] = area

    mats = _mats()
    in_maps = []
    for k in range(N_CORES):
        r0 = k * OWN
        in_maps.append({
            "bed": _to_dev(bedp[r0 : r0 + RS]),
            "press": _to_dev(pressp[r0 : r0 + RS]),
            "status": _to_dev(statusp[r0 : r0 + RS]),
            "melt": _to_dev(meltp[r0 : r0 + RQ]),
            "area": _to_dev(areap[r0 : r0 + RQ]),
            "conduit": _to_dev(cond[r0 : r0 + OWN]),
            "mats": mats,
        })
    return in_maps


def _from_dev(res_maps):
    out = np.empty((ROWS, COLS), np.float32)
    for k in range(N_CORES):
        g = res_maps[k]["grad"].reshape(P, NCH, OWN)    # [p, c, j]
        out[k * OWN : (k + 1) * OWN] = g.transpose(2, 0, 1).reshape(OWN, COLS)
    return out.ravel()


def run(inputs, trace=False, **kwargs):
    nc = _get_built()
    in_maps = _make_in_maps(
        inputs["melt_rate"], inputs["bedrock_elevation"],
        inputs["water_pressure"], inputs["cell_area"],
        inputs["conduit_size"], inputs["status_at_node"])
    res = run_bass_kernel_spmd(nc, in_maps, list(range(N_CORES)),
                               trace=trace, **kwargs)
    return _from_dev(res.results), res


def kernel(**inputs):
    out, _ = run(inputs)
    return out


# revision 13
# speedup vs baseline: 1.6782x; 1.0297x over previous
"""Trainium2 Bass kernel for nn_ConduitHydrology (MFD flow accumulation).

The reference graph is the raster 4-neighbor grid on a 1024x1024 raster, so
all segment_sums are 5-point stencil operations. Strategy:
  - Row-partition across 8 cores: core k owns global rows [128k, 128k+128),
    computing on a 192-row slab (32-row halo each side). 32 Jacobi
    iterations x 1-hop stencil => the halo fully absorbs cross-partition
    influence: zero inter-core communication.
  - On-chip layout (interleaved): column = p*8 + c for partition p, chunk
    c in [0,8); rows packed contiguously per chunk (f = c*192 + r for the
    q-domain, c*194 + r for the phi-domain). Row shifts and 7/8 of column
    shifts are free-dim offsets; only the chunk seam (c=7 <-> c=0 of the
    next partition) needs a partition-shift matmul.
  - Per iteration: 8 half-width fp16 products (DVE+GpSimd), 26 fp16
    matmuls on PE accumulating all shifted inflows into fp32 PSUM
    (24 of them with the identity as stationary), and 4 DVE adds
    (fp32 PSUM + fp32 runoff -> fp16 q). The last iteration assembles
    fp32 q for the output math.
The host only pads/slices/relayouts numpy arrays (no arithmetic on host).
"""

import numpy as np

import concourse.bass as bass
import concourse.mybir as mybir
from concourse.bacc import Bacc
from concourse.tile import TileContext
from concourse.bass_utils import run_bass_kernel_spmd

F32 = mybir.dt.float32
F16 = mybir.dt.bfloat16
I32 = mybir.dt.int32
ALU = mybir.AluOpType
ACTF = mybir.ActivationFunctionType

ROWS = COLS = 1024
N_CORES = 8
N_ITERS = 32
P = 128
NCH = 8
RQ = 192          # q-domain rows per slab
RS = 194          # phi-domain rows per slab
FQ = NCH * RQ     # 1536
FS = NCH * RS     # 1552
OWN = 128
OWN0 = 32

RHO_W, GRAV, SEC_PER_A = 1000.0, 9.81, 31556926.0
FLOW_COEFF = 0.0405
PAD_BED = 1.0e30


def build(n_iters=N_ITERS):
    nc = Bacc(None)

    bed_d = nc.declare_dram_parameter("bed", [P, FS], F32, isOutput=False)
    press_d = nc.declare_dram_parameter("press", [P, FS], F32, isOutput=False)
    status_d = nc.declare_dram_parameter("status", [P, FS], I32, isOutput=False)
    melt_d = nc.declare_dram_parameter("melt", [P, FQ], F32, isOutput=False)
    area_d = nc.declare_dram_parameter("area", [P, FQ], F32, isOutput=False)
    cond_d = nc.declare_dram_parameter("conduit", [P, 1024], F32, isOutput=False)
    mats_d = nc.declare_dram_parameter("mats", [P, 896], F32, isOutput=False)
    grad_d = nc.declare_dram_parameter("grad", [P, 1024], F32, isOutput=True)

    # phi-domain / q-domain chunk slices (1D)
    sch = lambda t, c, b, n: t[:, c * RS + b : c * RS + b + n]
    qch = lambda t, c, b, n: t[:, c * RQ + b : c * RQ + b + n]
    # 2D chunked views
    vs = lambda t, b, n: t.rearrange("p (c r) -> p c r", c=NCH)[:, :, b : b + n]
    vq = vs

    # iteration PSUM layout: chunk c at f = 512*(c//2) + 192*(c%2)
    pcf = lambda c: 512 * (c // 2) + 192 * (c % 2)
    # setup PSUM layout: chunk c at f = 256*c
    scf = lambda c: 256 * c

    with TileContext(nc) as tc:
        with (
            tc.tile_pool(name="main", bufs=1) as pool,
            tc.tile_pool(name="ps", bufs=2, space="PSUM") as pspool,
        ):
            def tmp(tag):
                return pool.tile([P, FS], F32, tag=tag, name=tag)

            def psum():
                return pspool.tile([P, 2048], F32, tag="ps", name="ps")

            def emit_group(ops):
                """ops: (out_ap, lhsT, rhs_ap, bank). start=True on the first
                matmul touching each PSUM bank (must cover the bank's used
                region), stop on the last."""
                last = {}
                for i, (o, w, rh, bank) in enumerate(ops):
                    last[bank] = i
                seen = set()
                for i, (o, w, rh, bank) in enumerate(ops):
                    st = bank not in seen
                    seen.add(bank)
                    nc.tensor.matmul(o, w, rh, start=st, stop=(last[bank] == i))

            # ---- constants
            mats = pool.tile([P, 896], F32)
            nc.sync.dma_start(out=mats[:], in_=mats_d[:])
            ID = mats[:, 0:128]
            SHD = mats[:, 128:256]   # out[m] = rhs[m-1]
            SHU = mats[:, 256:384]   # out[m] = rhs[m+1]
            EUP = mats[:, 512:640]   # out[127] = rhs[0]
            FIXC = mats[:, 640:896]  # row 0 = 1e33
            mats16 = pool.tile([P, 384], F16)
            nc.vector.tensor_copy(out=mats16[:], in_=mats[:, 0:384])
            ID16 = mats16[:, 0:128]
            SHD16 = mats16[:, 128:256]
            SHU16 = mats16[:, 256:384]

            # ---- inputs
            bed = tmp("t0")
            press = tmp("t1")
            status = pool.tile([P, FS], I32, tag="t2", name="t2")
            melt = tmp("t3")
            area = tmp("t4")
            cond = pool.tile([P, 1024], F32)
            for t, d, n in ((bed, bed_d, FS), (press, press_d, FS),
                            (status, status_d, FS), (melt, melt_d, FQ),
                            (area, area_d, FQ), (cond, cond_d, 1024)):
                nc.sync.dma_start(out=t[:, 0:n], in_=d[:])

            # ---- runoff (q-domain, fp32)
            r = pool.tile([P, FQ], F32)
            nc.vector.scalar_tensor_tensor(
                out=r[:], in0=melt[:, 0:FQ], scalar=1.0 / SEC_PER_A,
                in1=area[:, 0:FQ], op0=ALU.mult, op1=ALU.mult)

            # ---- potential and core mask (phi-domain)
            phi = tmp("t5")
            nc.vector.scalar_tensor_tensor(
                out=phi[:], in0=bed[:], scalar=RHO_W * GRAV,
                in1=press[:], op0=ALU.mult, op1=ALU.add)
            m = pool.tile([P, FS], F32)
            nc.vector.tensor_scalar(
                out=m[:], in0=status[:], scalar1=0, scalar2=None,
                op0=ALU.is_equal)

            # ---- E-neighbor phi / mask. E neighbor of (p,c): (p,c+1) for
            #      c<7, (p+1, chunk 0) for c=7 (seam); none at (p127,c7).
            def shift_from_east(dst, src, fix=None):
                ps = psum()
                ops = [(ps[:, scf(c) : scf(c) + RS], ID, sch(src, c + 1, 0, RS),
                        c // 2) for c in range(NCH - 1)]
                ops.append((ps[:, scf(7) : scf(7) + RS], SHU, sch(src, 0, 0, RS), 3))
                if fix is not None:
                    ops.append((ps[:, scf(7) : scf(7) + RS], EUP, fix[:, 0:RS], 3))
                emit_group(ops)
                nc.scalar.copy(vs(dst, 0, RS),
                               ps.rearrange("p (c r) -> p c r", c=8)[:, :, 0:RS])

            phiE = tmp("t3")
            shift_from_east(phiE, phi, fix=FIXC)
            mE = tmp("t4")
            shift_from_east(mE, m)

            # ---- directional drops (phi-domain link grids)
            dphiE = tmp("t0")
            nc.vector.tensor_sub(dphiE[:], phi[:], phiE[:])
            dropE = tmp("t1")    # flow col -> col+1, stored at col
            nc.vector.scalar_tensor_tensor(
                out=dropE[:], in0=dphiE[:], scalar=0.0, in1=m[:],
                op0=ALU.max, op1=ALU.mult)
            tw = tmp("t3")
            nc.vector.tensor_scalar(
                out=tw[:], in0=dphiE[:], scalar1=-1.0, scalar2=0.0,
                op0=ALU.mult, op1=ALU.max)
            dropW = pool.tile([P, FS], F32, tag="t2", name="t2f")
            nc.vector.tensor_mul(dropW[:], tw[:], mE[:])

            dphiS = tmp("t4")    # phi[r] - phi[r+1], link at r (per chunk)
            nc.vector.tensor_sub(vs(dphiS, 0, RS - 1), vs(phi, 0, RS - 1),
                                 vs(phi, 1, RS - 1))
            dropS = tmp("t6")    # flow r -> r+1, stored at r
            nc.vector.scalar_tensor_tensor(
                out=vs(dropS, 0, RS - 1), in0=vs(dphiS, 0, RS - 1), scalar=0.0,
                in1=vs(m, 0, RS - 1), op0=ALU.max, op1=ALU.mult)
            tn = tmp("t3")
            nc.vector.tensor_scalar(
                out=vs(tn, 0, RS - 1), in0=vs(dphiS, 0, RS - 1), scalar1=-1.0,
                scalar2=0.0, op0=ALU.mult, op1=ALU.max)
            dropN = tmp("t7")    # flow r+1 -> r, stored at r
            nc.vector.tensor_mul(vs(dropN, 0, RS - 1), vs(tn, 0, RS - 1),
                                 vs(m, 1, RS - 1))

            # ---- outgoing-W drop at its source (q-domain):
            #      dW[p,c] = dropW[(p,c-1)] | dropW[(p-1, c7)]
            psW = psum()
            ops = [(psW[:, scf(c) : scf(c) + RQ], ID, sch(dropW, c - 1, 1, RQ),
                    c // 2) for c in range(1, NCH)]
            ops.append((psW[:, scf(0) : scf(0) + RQ], SHD, sch(dropW, 7, 1, RQ), 0))
            emit_group(ops)
            dW = pool.tile([P, FQ], F32, tag="t3", name="t3w")
            nc.scalar.copy(vq(dW, 0, RQ),
                           psW.rearrange("p (c r) -> p c r", c=8)[:, :, 0:RQ])

            # ---- total outgoing drop (q-domain)
            psT = psum()
            ops = []
            for c in range(NCH):
                o = psT[:, scf(c) : scf(c) + RQ]
                ops += [(o, ID, sch(dropE, c, 1, RQ), c // 2),
                        (o, ID, sch(dropS, c, 1, RQ), c // 2),
                        (o, ID, sch(dropN, c, 0, RQ), c // 2),
                        (o, ID, qch(dW, c, 0, RQ), c // 2)]
            emit_group(ops)
            tds = pool.tile([P, FQ], F32, tag="t0", name="t0t")
            nc.vector.tensor_scalar(
                out=vq(tds, 0, RQ),
                in0=psT.rearrange("p (c r) -> p c r", c=8)[:, :, 0:RQ],
                scalar1=1.0e-30, scalar2=None, op0=ALU.max)
            recip = pool.tile([P, FQ], F32, tag="t4", name="t4r")
            nc.vector.reciprocal(recip[:], tds[:])

            # ---- outflow fractions, cast to fp16 (q-domain, source node)
            fE = pool.tile([P, FQ], F16)
            fW = pool.tile([P, FQ], F16)
            fS = pool.tile([P, FQ], F16)
            fN = pool.tile([P, FQ], F16)
            nc.vector.tensor_mul(vq(fE, 0, RQ), vs(dropE, 1, RQ), vq(recip, 0, RQ))
            nc.vector.tensor_mul(fW[:], dW[:], recip[:])
            nc.vector.tensor_mul(vq(fS, 0, RQ), vs(dropS, 1, RQ), vq(recip, 0, RQ))
            nc.vector.tensor_mul(vq(fN, 0, RQ), vs(dropN, 0, RQ), vq(recip, 0, RQ))

            # slab-edge outflow rows leave the slab; zero them so the
            # pair-merged row-shift matmuls bleed exact zeros across the
            # chunk boundary inside each PSUM bank.
            nc.vector.memset(vq(fS, RQ - 1, 1), 0.0)
            nc.vector.memset(vq(fN, 0, 1), 0.0)

            # ---- discharge iteration state
            q16 = pool.tile([P, FQ], F16)
            nc.scalar.copy(q16[:], r[:])
            q32 = pool.tile([P, FQ], F32)
            oE = pool.tile([P, FQ], F16)
            oW = pool.tile([P, FQ], F16)
            oS = pool.tile([P, FQ], F16)
            oN = pool.tile([P, FQ], F16)

            H = FQ // 2
            for it in range(n_iters):
                lastit = it == n_iters - 1
                # products. DVE: oW/oE at pair granularity, ordered so the
                # bank-0 seam operand (oE pair 3) is ready early; GpSimd
                # (slower, ~2.5 cyc/elem floor) gets 3 halves of oS/oN and
                # DVE absorbs the last.
                PR = 384
                for pr in (0, 1, 2, 3):
                    sl = slice(pr * PR, (pr + 1) * PR)
                    nc.vector.tensor_mul(oW[:, sl], fW[:, sl], q16[:, sl])
                for pr in (3, 0, 1, 2):
                    sl = slice(pr * PR, (pr + 1) * PR)
                    nc.vector.tensor_mul(oE[:, sl], fE[:, sl], q16[:, sl])
                nc.gpsimd.tensor_mul(oS[:, 0:H], fS[:, 0:H], q16[:, 0:H])
                nc.gpsimd.tensor_mul(oN[:, 0:H], fN[:, 0:H], q16[:, 0:H])
                nc.gpsimd.tensor_mul(oS[:, H:FQ], fS[:, H:FQ], q16[:, H:FQ])
                nc.vector.tensor_mul(oN[:, H:FQ], fN[:, H:FQ], q16[:, H:FQ])

                ps = psum()
                qdst = q32 if lastit else q16
                # Per-bank, in order: starter (covers the bank's whole used
                # region), accumulators, then the q assembly for that bank
                # so DVE drains banks while PE works on later ones.
                bank_ops = [
                    [   # bank 0: chunks 0,1
                        (ps[:, 0:384], ID16, oW[:, 192:576], 0),
                        (ps[:, 192:384], ID16, oE[:, 0:192], 0),
                        (ps[:, 0:192], SHD16, oE[:, 1344:1536], 0),
                        (ps[:, 1:384], ID16, oS[:, 0:383], 0),
                        (ps[:, 0:383], ID16, oN[:, 1:384], 0),
                    ],
                    [   # bank 1: chunks 2,3
                        (ps[:, 512:896], ID16, oW[:, 576:960], 1),
                        (ps[:, 512:896], ID16, oE[:, 192:576], 1),
                        (ps[:, 513:896], ID16, oS[:, 384:767], 1),
                        (ps[:, 512:895], ID16, oN[:, 385:768], 1),
                    ],
                    [   # bank 2: chunks 4,5
                        (ps[:, 1024:1408], ID16, oW[:, 960:1344], 2),
                        (ps[:, 1024:1408], ID16, oE[:, 576:960], 2),
                        (ps[:, 1025:1408], ID16, oS[:, 768:1151], 2),
                        (ps[:, 1024:1407], ID16, oN[:, 769:1152], 2),
                    ],
                    [   # bank 3: chunks 6,7
                        (ps[:, 1536:1920], ID16, oE[:, 960:1344], 3),
                        (ps[:, 1536:1728], ID16, oW[:, 1344:1536], 3),
                        (ps[:, 1728:1920], SHU16, oW[:, 0:192], 3),
                        (ps[:, 1537:1920], ID16, oS[:, 1152:1535], 3),
                        (ps[:, 1536:1919], ID16, oN[:, 1153:1536], 3),
                    ],
                ]
                for b in range(4):
                    for i, (o, w, rh, _bk) in enumerate(bank_ops[b]):
                        nc.tensor.matmul(o, w, rh, start=(i == 0),
                                         stop=(i == len(bank_ops[b]) - 1))
                    nc.vector.tensor_add(
                        out=qdst[:, 384 * b : 384 * b + 384],
                        in0=ps[:, 512 * b : 512 * b + 384],
                        in1=r[:, 384 * b : 384 * b + 384])

            # ---- gradient on owned rows (compact [p, c*128+j] layout)
            s1 = pool.tile([P, 1024], F32, tag="f0", name="f0")
            nc.scalar.sqrt(s1[:], cond[:])
            s2 = pool.tile([P, 1024], F32, tag="f1", name="f1")
            nc.scalar.sqrt(s2[:], s1[:])
            c125 = pool.tile([P, 1024], F32, tag="f0", name="f0b")
            nc.vector.tensor_mul(c125[:], cond[:], s2[:])
            k0 = pool.tile([P, 1024], F32, tag="f1", name="f1b")
            nc.scalar.activation(k0[:], c125[:], ACTF.Square,
                                 scale=float(FLOW_COEFF))
            vo = lambda t: t.rearrange("p (c j) -> p c j", c=NCH)
            km = pool.tile([P, 1024], F32, tag="f0", name="f0c")
            nc.vector.tensor_mul(vo(km), vo(k0), vs(m, OWN0 + 1, OWN))
            q2 = pool.tile([P, 1024], F32, tag="f1", name="f1c")
            nc.scalar.activation(vo(q2), vq(q32, OWN0, OWN), ACTF.Square)
            g = pool.tile([P, 1024], F32, tag="f2", name="f2")
            nc.vector.tensor_mul(g[:], q2[:], km[:])

            nc.sync.dma_start(out=grad_d[:], in_=g[:])

    nc.finalize()
    return nc


# ------------------------------------------------------------------ host side

def _mats():
    ident = np.eye(P, dtype=np.float32)
    shd = np.zeros((P, P), np.float32)
    shd[np.arange(P - 1), np.arange(1, P)] = 1.0      # out[m] = rhs[m-1]
    shu = np.zeros((P, P), np.float32)
    shu[np.arange(1, P), np.arange(P - 1)] = 1.0      # out[m] = rhs[m+1]
    edn = np.zeros((P, P), np.float32)
    edn[P - 1, 0] = 1.0
    eup = np.zeros((P, P), np.float32)
    eup[0, P - 1] = 1.0
    fixc = np.zeros((P, 2 * P), np.float32)
    fixc[0, :] = 1.0e33
    return np.concatenate([ident, shd, shu, edn, eup, fixc], axis=1)


def _to_dev(slab):
    """[rows, 1024] row-major slab -> [128, 8*rows], col = p*8 + c."""
    rows = slab.shape[0]
    return np.ascontiguousarray(
        slab.reshape(rows, P, NCH).transpose(1, 2, 0)).reshape(P, NCH * rows)


_BUILT = None


def _get_built():
    global _BUILT
    if _BUILT is None:
        _BUILT = build()
    return _BUILT


def _make_in_maps(melt_rate, bedrock_elevation, water_pressure, cell_area,
                  conduit_size, status_at_node):
    grid = lambda a: np.asarray(a).reshape(ROWS, COLS)
    bed = grid(bedrock_elevation).astype(np.float32)
    press = grid(water_pressure).astype(np.float32)
    status = grid(status_at_node).astype(np.int32)
    melt = grid(melt_rate).astype(np.float32)
    area = grid(cell_area).astype(np.float32)
    cond = grid(conduit_size).astype(np.float32)

    gp = 33
    bedp = np.full((ROWS + 2 * gp, COLS), PAD_BED, np.float32)
    bedp[gp:gp + ROWS] = bed
    pressp = np.zeros((ROWS + 2 * gp, COLS), np.float32)
    pressp[gp:gp + ROWS] = press
    statusp = np.ones((ROWS + 2 * gp, COLS), np.int32)
    statusp[gp:gp + ROWS] = status
    gq = 32
    meltp = np.zeros((ROWS + 2 * gq, COLS), np.float32)
    meltp[gq:gq + ROWS] = melt
    areap = np.zeros((ROWS + 2 * gq, COLS), np.float32)
    areap[gq:gq + ROWS] = area

    mats = _mats()
    in_maps = []
    for k in range(N_CORES):
        r0 = k * OWN
        in_maps.append({
            "bed": _to_dev(bedp[r0 : r0 + RS]),
            "press": _to_dev(pressp[r0 : r0 + RS]),
            "status": _to_dev(statusp[r0 : r0 + RS]),
            "melt": _to_dev(meltp[r0 : r0 + RQ]),
            "area": _to_dev(areap[r0 : r0 + RQ]),
            "conduit": _to_dev(cond[r0 : r0 + OWN]),
            "mats": mats,
        })
    return in_maps


def _from_dev(res_maps):
    out = np.empty((ROWS, COLS), np.float32)
    for k in range(N_CORES):
        g = res_maps[k]["grad"].reshape(P, NCH, OWN)    # [p, c, j]
        out[k * OWN : (k + 1) * OWN] = g.transpose(2, 0, 1).reshape(OWN, COLS)
    return out.ravel()


def run(inputs, trace=False, **kwargs):
    nc = _get_built()
    in_maps = _make_in_maps(
        inputs["melt_rate"], inputs["bedrock_elevation"],
        inputs["water_pressure"], inputs["cell_area"],
        inputs["conduit_size"], inputs["status_at_node"])
    res = run_bass_kernel_spmd(nc, in_maps, list(range(N_CORES)),
                               trace=trace, **kwargs)
    return _from_dev(res.results), res


def kernel(**inputs):
    out, _ = run(inputs)
    return out
